# revision 2
# baseline (speedup 1.0000x reference)
"""GAT layer (gnn_message_passing) Trainium2 Bass kernel, 8-core SPMD. v3

Strategy
--------
Nodes are partitioned contiguously across the 8 cores (6272 nodes each);
since dst is sorted, each core owns the contiguous run of edges whose dst
falls in its node range and computes those output rows fully locally.

v3 has NO table-build phase: edges gather raw h rows [N, 128] f16
directly with dma_gather(transpose=True), which lands FEATURES on
partitions ([128, 1, E]). A per-slot PE matmul with the stationary
rhs_aug = [asrc_eff | fc_w.T] then computes Z = [s_src | z] for the
slot's 128 edges -- contraction over features puts EDGES back on
partitions, exactly what the downstream scatter needs. This replaces
the v1/v2 z-table in DRAM (25.6MB of build+write traffic and a
table-before-gather barrier) with 1.66 GFLOP/core of PE work.

HW-validated constraints for the transposed gather (found the hard
way): (a) all transposed gathers must ride ONE SWDGE queue --
concurrent transposed gathers on multiple queues race on the xbar and
corrupt ~15% of outputs; (b) num_idxs per call is capped at 896 (1024
hard-crashes the device); (c) negative gather indices fetch
nondeterministic garbage (zeros / wrong rows), so the int16 index
range forces the lo/hi table split at 32768.

Per batch (8 windows, ~70 slots):
  gather h_src (transposed)           [SWDGE, GCAP slots/call]
  per slot: Z[128e, 65] = g.T @ rhs   [PE, psZ 8-slot PSUM tiles]
  per 8-slot group: zc <- Z (f16)     [ACT copy PSUM->SBUF]
  e = zc[:,:,0] + s_dst; w = exp(lrelu(e))   [DVE + ACT exp]
  zc[:,:,0] <- w (ACT); zc[:,:,1:65] *= w    [DVE, in place]
  per slot: PSUM[W, wi, 0:65] += P_slot.T @ zc_slot   [PE]
  finalize per batch: den/recip/mul over all windows  [DVE]
with the one-hot P built on DVE (dl vs iota compare), PREB batches
ahead. Edge bookkeeping (dl, s_dst, int16 gather idx in lo/hi halves
split at 32768) is prepared on host and shipped in packed tensors.
"""

import os
import numpy as np

N_NODES = 50000
N_EDGES = 800000
IN_DIM = 128
OUT_DIM = 64
NEG_SLOPE = 0.01
NCORES = 8
W = 64           # nodes per window
TILE = 128       # edges per tile (= matmul contraction)
SPLIT = 32768    # int16 gather index limit
B_WIN = 8        # windows per batch
N_SH = 6272      # nodes per core (= 8*6272 = 50176 >= 50000)
N_PAD = NCORES * N_SH  # 50176
NW = N_SH // W   # 98 windows per core
ZGRP = 8         # slots per PSUM Z tile / ACT copy group
GCAP = int(os.environ.get("KERNEL_GCAP", "7"))   # slots per dma_gather
SCRATCH = int(os.environ.get("KERNEL_SCRATCH", "16384"))
NQ = int(os.environ.get("KERNEL_NQ", "1"))       # swdge queues used

_F16 = np.float16
_F32 = np.float32

LAST_EXEC_NS = None


# ----------------------------------------------------------------------
# Host planning
# ----------------------------------------------------------------------

def _plan(src, dst):
    splits = [int(np.searchsorted(dst, c * N_SH, side="left"))
              for c in range(NCORES + 1)]

    # per-window tile counts, shared across cores (SPMD: one NEFF)
    tlo = np.zeros(NW, dtype=np.int64)
    thi = np.zeros(NW, dtype=np.int64)
    for c in range(NCORES):
        s, e = splits[c], splits[c + 1]
        win = (dst[s:e] - c * N_SH) // W
        lo = src[s:e] < SPLIT
        wlo = np.bincount(win, weights=lo.astype(np.float64), minlength=NW).astype(np.int64)
        whi = np.bincount(win, minlength=NW) - wlo
        np.maximum(tlo, -(-wlo // TILE), out=tlo)
        np.maximum(thi, -(-whi // TILE), out=thi)
    tlo = np.maximum(tlo, 1)
    thi = np.maximum(thi, 1)

    off_lo = np.concatenate([[0], np.cumsum(tlo)])
    off_hi = np.concatenate([[0], np.cumsum(thi)])

    batches = [list(range(b, min(b + B_WIN, NW))) for b in range(0, NW, B_WIN)]

    S_LO, S_HI = int(off_lo[-1]), int(off_hi[-1])
    # packed [128, *] f16 column layout (no hT here -- h rides row-major
    # in its own "hrow" tensor for the transposed gather)
    pk = {}
    o = 0
    for name, w_ in (("rhs", 65), ("iota", W),
                     ("dl_lo", S_LO), ("dl_hi", S_HI),
                     ("sd_lo", S_LO), ("sd_hi", S_HI)):
        pk[name] = o
        o += w_
    pk["end"] = o

    return dict(
        splits=splits,
        tlo=tlo, thi=thi, off_lo=off_lo, off_hi=off_hi,
        S_LO=S_LO, S_HI=S_HI, batches=batches, pk=pk,
    )


def _wrap_idx(arr16):
    """[S*128] int16 -> gather idx layout [16, S*8] (i%16 part, i//16 col).
    The 8x replication across 16-partition groups happens on-device."""
    return arr16.reshape(-1, 16).T.copy()  # [16, S*8]


def _build_core_inputs(c, plan, src, dst, s_dst_node):
    s, e = plan["splits"][c], plan["splits"][c + 1]
    n0 = c * N_SH
    src_c = src[s:e]
    dst_c = dst[s:e]
    dloc = dst_c - n0
    win = dloc // W
    S_LO, S_HI = plan["S_LO"], plan["S_HI"]
    off_lo, off_hi = plan["off_lo"], plan["off_hi"]

    idx_lo = np.zeros(S_LO * TILE, dtype=np.int16)
    idx_hi = np.zeros(S_HI * TILE, dtype=np.int16)
    dstloc = np.full((S_LO + S_HI) * TILE, 999.0, dtype=_F16)
    sdst = np.zeros((S_LO + S_HI) * TILE, dtype=_F16)

    for half in ("lo", "hi"):
        mask = src_c < SPLIT if half == "lo" else src_c >= SPLIT
        ei = np.nonzero(mask)[0]
        w_e = win[ei]
        starts = np.searchsorted(w_e, np.arange(NW + 1))
        rank = np.arange(len(ei)) - starts[w_e]
        off = off_lo if half == "lo" else off_hi
        base = 0 if half == "lo" else S_LO * TILE
        flat = base + off[w_e] * TILE + rank
        dstloc[flat] = (dloc[ei] % W).astype(_F16)
        sdst[flat] = s_dst_node[dst_c[ei]]
        if half == "lo":
            idx_lo[off[w_e] * TILE + rank] = src_c[ei].astype(np.int16)
        else:
            idx_hi[off[w_e] * TILE + rank] = (src_c[ei] - SPLIT).astype(np.int16)

    dl2 = dstloc.reshape(S_LO + S_HI, TILE).T  # [128, S_LO+S_HI]
    sd2 = sdst.reshape(S_LO + S_HI, TILE).T

    pack_edge = np.empty((128, 2 * (S_LO + S_HI)), dtype=_F16)
    pack_edge[:, 0:S_LO] = dl2[:, :S_LO]
    pack_edge[:, S_LO:S_LO + S_HI] = dl2[:, S_LO:]
    pack_edge[:, S_LO + S_HI:2 * S_LO + S_HI] = sd2[:, :S_LO]
    pack_edge[:, 2 * S_LO + S_HI:] = sd2[:, S_LO:]

    pack16 = np.concatenate([_wrap_idx(idx_lo), _wrap_idx(idx_hi)], axis=1)
    return pack_edge, pack16


def _host_prep(h, src, dst, fc_w, attn_w):
    """Shared host-side preprocessing -> (plan, in_maps)."""
    plan = _plan(src, dst)

    a_src = attn_w[0, :OUT_DIM]
    a_dst = attn_w[0, OUT_DIM:]
    asrc_eff = fc_w.T @ a_src          # [128]
    adst_eff = fc_w.T @ a_dst          # [128]
    s_dst_node = (h @ adst_eff).astype(_F16)  # [N] host-side expansion data

    hrow = np.zeros((N_PAD, IN_DIM), dtype=_F16)
    hrow[:N_NODES] = h.astype(_F16)
    rhs_aug = np.concatenate(
        [asrc_eff[:, None], fc_w.T], axis=1).astype(_F16)  # [128, 65]
    iota_row = np.tile(np.arange(W, dtype=_F16)[None, :], (128, 1))
    pk = plan["pk"]

    common = np.empty((128, pk["dl_lo"]), dtype=_F16)
    common[:, pk["rhs"]:pk["rhs"] + 65] = rhs_aug
    common[:, pk["iota"]:pk["iota"] + W] = iota_row

    in_maps = []
    for c in range(NCORES):
        pack_edge, pack16 = _build_core_inputs(c, plan, src, dst, s_dst_node)
        pack = np.concatenate([common, pack_edge], axis=1)
        in_maps.append({"hrow": hrow.view(np.int16),
                        "pack": pack.view(np.int16), "pack16": pack16})
    return plan, in_maps


# ----------------------------------------------------------------------
# Bass program
# ----------------------------------------------------------------------

def _build_bass(plan):
    import concourse.bacc as bacc
    import concourse.mybir as mybir
    import concourse.tile as tile

    f16 = mybir.dt.float16
    f32 = mybir.dt.float32
    i16 = mybir.dt.int16

    S_LO, S_HI = plan["S_LO"], plan["S_HI"]
    tlo, thi = plan["tlo"], plan["thi"]
    off_lo, off_hi = plan["off_lo"], plan["off_hi"]
    pk = plan["pk"]

    nc = bacc.Bacc("TRN2", target_bir_lowering=False, debug=False,
                   num_swdge_queues=4, dynamic_dma_scratch_size=SCRATCH)

    hrow_d = nc.declare_dram_parameter("hrow", [N_PAD, IN_DIM], i16, isOutput=False)
    pack_d = nc.declare_dram_parameter("pack", [128, pk["end"]], i16, isOutput=False)
    pack16_d = nc.declare_dram_parameter("pack16", [16, (S_LO + S_HI) * 8], i16, isOutput=False)
    hout = nc.declare_dram_parameter("hout", [N_SH, OUT_DIM], f16, isOutput=True)
    packf = pack_d[:].bitcast(f16)
    hrowf = hrow_d[:].bitcast(f16)

    tab_lo = hrowf[0:SPLIT, :]
    tab_hi = hrowf[SPLIT:N_PAD, :]

    nbatch = int(os.environ.get("KERNEL_NBATCH", "0")) or len(plan["batches"])

    with tile.TileContext(nc) as tc:
        with (
            tc.tile_pool(name="sbB", bufs=3) as sbB,
            tc.tile_pool(name="sbP", bufs=5) as sbP,
            tc.tile_pool(name="sbBs", bufs=1) as sbBs,
            tc.tile_pool(name="sbC", bufs=3) as sbC,
            tc.tile_pool(name="psZ", bufs=2, space="PSUM") as psZ,
            tc.tile_pool(name="psW", bufs=2, space="PSUM") as psW,
        ):
            rhs_t = sbBs.tile([128, 65], f16, tag="rhs")
            nc.sync.dma_start(rhs_t[:], packf[:, pk["rhs"]:pk["rhs"] + 65])
            iota_t = sbBs.tile([128, W], f16, tag="iota")
            nc.sync.dma_start(iota_t[:], packf[:, pk["iota"]:pk["iota"] + W])
            # per-edge dst-local ids and s_dst, whole slot grid upfront
            dl_t = sbBs.tile([128, S_LO + S_HI], f16, tag="dl")
            nc.scalar.dma_start(dl_t[:, :S_LO], packf[:, pk["dl_lo"]:pk["dl_lo"] + S_LO])
            nc.scalar.dma_start(dl_t[:, S_LO:], packf[:, pk["dl_hi"]:pk["dl_hi"] + S_HI])
            sd_t = sbBs.tile([128, S_LO + S_HI], f16, tag="sd")
            nc.scalar.dma_start(sd_t[:, :S_LO], packf[:, pk["sd_lo"]:pk["sd_lo"] + S_LO])
            nc.scalar.dma_start(sd_t[:, S_LO:], packf[:, pk["sd_hi"]:pk["sd_hi"] + S_HI])
            # gather indices: ship [16, S*8] once, replicate into the 8
            # 16-partition groups on-device (SWDGE reads all 128 parts)
            it_t = sbBs.tile([128, (S_LO + S_HI) * 8], i16, tag="it")
            for g in range(8):
                nc.sync.dma_start(it_t[16 * g:16 * (g + 1), :], pack16_d[:])
            IT_HI0 = S_LO * 8  # column offset of hi idxs within it_t

            # Tile assigns DMASW sem lanes round-robin in emission order;
            # queue_num must track it so lane L always pairs queue L%4.
            gather_counter = [0]

            def batch_meta(bi):
                wins = plan["batches"][bi]
                alo = int(off_lo[wins[0]])
                ahi = int(off_hi[wins[0]])
                n_lo = int(off_lo[wins[-1] + 1]) - alo
                n_hi = int(off_hi[wins[-1] + 1]) - ahi
                return wins, alo, ahi, n_lo, n_hi

            # one-hot P depends only on dl_t/iota: emit builds PREB batches
            # ahead so DVE stays ahead of the PE/gather pipeline.
            PREB = 5
            P_tiles = {}

            def build_P(bi):
                _, alo, ahi, n_lo, n_hi = batch_meta(bi)
                n = n_lo + n_hi
                P = sbP.tile([128, n, W], f16, tag="P")
                nc.vector.tensor_tensor(
                    P[:, :n_lo, :],
                    dl_t[:, alo:alo + n_lo, None].to_broadcast([128, n_lo, W]),
                    iota_t[:, None, :].to_broadcast([128, n_lo, W]),
                    op=mybir.AluOpType.is_equal)
                nc.vector.tensor_tensor(
                    P[:, n_lo:, :],
                    dl_t[:, S_LO + ahi:S_LO + ahi + n_hi, None].to_broadcast(
                        [128, n_hi, W]),
                    iota_t[:, None, :].to_broadcast([128, n_hi, W]),
                    op=mybir.AluOpType.is_equal)
                P_tiles[bi] = P

            for bi in range(min(PREB, nbatch)):
                build_P(bi)

            for bi, wins in enumerate(plan["batches"][:nbatch]):
                w0 = wins[0]
                nb = len(wins)
                _, alo, ahi, n_lo, n_hi = batch_meta(bi)
                n = n_lo + n_hi

                # transposed gather: features on partitions, edges on free
                g2 = sbB.tile([128, n * TILE], f16, tag="g")
                for half, tab, a, nh, gbase, itbase in (
                    ("lo", tab_lo, alo, n_lo, 0, 0),
                    ("hi", tab_hi, ahi, n_hi, n_lo, IT_HI0),
                ):
                    for o in range(0, nh, GCAP):
                        k = min(GCAP, nh - o)
                        nc.gpsimd.dma_gather(
                            g2[:, (gbase + o) * TILE:(gbase + o + k) * TILE]
                            .rearrange("p (u e) -> p u e", u=1),
                            tab[:],
                            it_t[:, itbase + (a + o) * 8:itbase + (a + o + k) * 8],
                            num_idxs=k * TILE, num_idxs_reg=k * TILE,
                            elem_size=128, transpose=True,
                            queue_num=gather_counter[0] % NQ,
                        )
                        gather_counter[0] += 1

                # per-slot z matmul + PSUM->SBUF copy in ZGRP groups
                zc = sbB.tile([128, n, 65], f16, tag="zc")
                for z0 in range(0, n, ZGRP):
                    zn = min(ZGRP, n - z0)
                    Z = psZ.tile([128, ZGRP, 128], f32, tag="Z")
                    for si in range(zn):
                        nc.tensor.matmul(
                            Z[:, si, 0:65],
                            lhsT=g2[:, (z0 + si) * TILE:(z0 + si + 1) * TILE],
                            rhs=rhs_t[:], start=True, stop=True)
                    nc.scalar.copy(zc[:, z0:z0 + zn, :], Z[:, 0:zn, 0:65])

                # w = exp(leaky_relu(s_src + s_dst)); zc <- [w | w*z]
                e_t = sbB.tile([128, n], f16, tag="e")
                nc.vector.tensor_tensor(e_t[:, :n_lo], zc[:, :n_lo, 0],
                                        sd_t[:, alo:alo + n_lo],
                                        op=mybir.AluOpType.add)
                nc.vector.tensor_tensor(e_t[:, n_lo:], zc[:, n_lo:, 0],
                                        sd_t[:, S_LO + ahi:S_LO + ahi + n_hi],
                                        op=mybir.AluOpType.add)
                es = sbB.tile([128, n], f16, tag="es")
                nc.vector.tensor_scalar_mul(es[:], e_t[:], NEG_SLOPE)
                el = sbB.tile([128, n], f16, tag="el")
                nc.vector.tensor_tensor(el[:], e_t[:], es[:],
                                        op=mybir.AluOpType.max)
                wt = sbB.tile([128, n], f16, tag="w")
                nc.scalar.activation(wt[:], el[:],
                                     mybir.ActivationFunctionType.Exp)
                nc.scalar.copy(zc[:, :, 0:1], wt[:, :, None])
                nc.vector.tensor_tensor(
                    zc[:, :, 1:65], zc[:, :, 1:65],
                    wt[:, :, None].to_broadcast([128, n, 64]),
                    op=mybir.AluOpType.mult)
                P = P_tiles.pop(bi)

                ps = psW.tile([W, nb, 128], f32, tag="win")
                for wi, wv in enumerate(wins):
                    nmm = int(tlo[wv] + thi[wv])
                    k = 0
                    for j in range(int(tlo[wv])):
                        s_rel = int(off_lo[wv]) - alo + j
                        nc.tensor.matmul(ps[:, wi, 0:65], lhsT=P[:, s_rel, :],
                                         rhs=zc[:, s_rel, :],
                                         start=(k == 0), stop=(k == nmm - 1))
                        k += 1
                    for j in range(int(thi[wv])):
                        s_rel = n_lo + int(off_hi[wv]) - ahi + j
                        nc.tensor.matmul(ps[:, wi, 0:65], lhsT=P[:, s_rel, :],
                                         rhs=zc[:, s_rel, :],
                                         start=(k == 0), stop=(k == nmm - 1))
                        k += 1

                den = sbC.tile([W, nb], f32, tag="den")
                nc.vector.tensor_scalar_max(den[:], ps[:, :, 0], 1e-30)
                rec = sbC.tile([W, nb], f32, tag="rec")
                nc.vector.reciprocal(rec[:], den[:])
                ho = sbC.tile([W, nb * OUT_DIM], f16, tag="ho")
                nc.vector.tensor_tensor(
                    ho[:].rearrange("p (b c) -> p b c", b=nb),
                    ps[:, :, 1:65],
                    rec[:, :, None].to_broadcast([W, nb, OUT_DIM]),
                    op=mybir.AluOpType.mult)

                out_view = hout[w0 * W:(w0 + nb) * W, :].rearrange(
                    "(b p) c -> p b c", p=W)
                nc.sync.dma_start(
                    out_view,
                    ho[:].rearrange("p (b c) -> p b c", b=nb))

                if bi + PREB < nbatch:
                    build_P(bi + PREB)

    if not nc.is_finalized():
        nc.finalize()
    return nc


# ----------------------------------------------------------------------
# Entry point
# ----------------------------------------------------------------------

def kernel(h, src, dst, fc_w, attn_w):
    from concourse.bass_utils import run_bass_kernel_spmd

    h = np.asarray(h, dtype=_F32)
    src = np.asarray(src, dtype=np.int32)
    dst = np.asarray(dst, dtype=np.int32)
    fc_w = np.asarray(fc_w, dtype=_F32)
    attn_w = np.asarray(attn_w, dtype=_F32)

    plan, in_maps = _host_prep(h, src, dst, fc_w, attn_w)

    nc = _build_bass(plan)
    res = run_bass_kernel_spmd(nc, in_maps, list(range(NCORES)))
    global LAST_EXEC_NS
    LAST_EXEC_NS = res.exec_time_ns

    full = np.concatenate(
        [res.results[c]["hout"] for c in range(NCORES)], axis=0)
    return full[:N_NODES].astype(_F32)


# revision 3
# speedup vs baseline: 1.1749x; 1.1749x over previous
"""GAT layer (gnn_message_passing) Trainium2 Bass kernel, 8-core SPMD. v6

Strategy
--------
Nodes are partitioned contiguously across the 8 cores (6272 nodes each);
since dst is sorted, each core owns the contiguous run of edges whose dst
falls in its node range and computes those output rows fully locally.

v3 has NO table-build phase: edges gather raw h rows [N, 128] f16
directly with dma_gather(transpose=True), which lands FEATURES on
partitions ([128, 1, E]). A per-slot PE matmul with the stationary
rhs_aug = [asrc_eff | fc_w.T] then computes Z = [s_src | z] for the
slot's 128 edges -- contraction over features puts EDGES back on
partitions, exactly what the downstream scatter needs. This replaces
the v1/v2 z-table in DRAM (25.6MB of build+write traffic and a
table-before-gather barrier) with 1.66 GFLOP/core of PE work.

HW-validated constraints for the transposed gather (found the hard
way): (a) all transposed gathers must ride ONE SWDGE queue --
concurrent transposed gathers on multiple queues race on the xbar and
corrupt ~15% of outputs; (b) num_idxs per call is capped at 896 (1024
hard-crashes the device); (c) negative gather indices fetch
nondeterministic garbage (zeros / wrong rows), so the int16 index
range forces the lo/hi table split at 32768.

Per batch (8 windows, ~70 slots):
  gather h_src (transposed)           [SWDGE, GCAP slots/call]
  per slot: Z[128e, 65] = g.T @ rhs   [PE, psZ 8-slot PSUM tiles]
  per 8-slot group: zc <- Z (f16)     [ACT copy PSUM->SBUF]
  e = zc[:,:,0] + s_dst; w = exp(lrelu(e))   [DVE + ACT exp]
  zc[:,:,0] <- w (ACT); zc[:,:,1:65] *= w    [DVE, in place]
  per slot: PSUM[W, wi, 0:65] += P_slot.T @ zc_slot   [PE]
  finalize per batch: den/recip/mul over all windows  [DVE]
with the one-hot P built on DVE (dl vs iota compare), PREB batches
ahead. Edge bookkeeping (dl, s_dst, int16 gather idx in lo/hi halves
split at 32768) is prepared on host and shipped in packed tensors.
"""

import os
import numpy as np

N_NODES = 50000
N_EDGES = 800000
IN_DIM = 128
OUT_DIM = 64
NEG_SLOPE = 0.01
NCORES = 8
W = 64           # nodes per window
TILE = 128       # edges per tile (= matmul contraction)
SPLIT = 32768    # int16 gather index limit
B_WIN = 8        # windows per batch
N_SH = 6272      # nodes per core (= 8*6272 = 50176 >= 50000)
N_PAD = NCORES * N_SH  # 50176
NW = N_SH // W   # 98 windows per core
ZGRP = 8         # slots per PSUM Z tile / ACT copy group
GCAP = int(os.environ.get("KERNEL_GCAP", "7"))   # slots per dma_gather
SCRATCH = int(os.environ.get("KERNEL_SCRATCH", "16384"))
NQ = int(os.environ.get("KERNEL_NQ", "1"))       # swdge queues used

_F16 = np.float16
_F32 = np.float32

LAST_EXEC_NS = None


# ----------------------------------------------------------------------
# Host planning
# ----------------------------------------------------------------------

def _plan(src, dst):
    splits = [int(np.searchsorted(dst, c * N_SH, side="left"))
              for c in range(NCORES + 1)]

    # per-window tile counts, shared across cores (SPMD: one NEFF)
    tlo = np.zeros(NW, dtype=np.int64)
    thi = np.zeros(NW, dtype=np.int64)
    for c in range(NCORES):
        s, e = splits[c], splits[c + 1]
        win = (dst[s:e] - c * N_SH) // W
        lo = src[s:e] < SPLIT
        wlo = np.bincount(win, weights=lo.astype(np.float64), minlength=NW).astype(np.int64)
        whi = np.bincount(win, minlength=NW) - wlo
        np.maximum(tlo, -(-wlo // TILE), out=tlo)
        np.maximum(thi, -(-whi // TILE), out=thi)
    tlo = np.maximum(tlo, 1)
    thi = np.maximum(thi, 1)

    off_lo = np.concatenate([[0], np.cumsum(tlo)])
    off_hi = np.concatenate([[0], np.cumsum(thi)])

    batches = [list(range(b, min(b + B_WIN, NW))) for b in range(0, NW, B_WIN)]

    S_LO, S_HI = int(off_lo[-1]), int(off_hi[-1])
    # packed [128, *] f16 column layout (no hT here -- h rides row-major
    # in its own "hrow" tensor for the transposed gather)
    pk = {}
    o = 0
    for name, w_ in (("rhs", 65), ("iota", W),
                     ("dl_lo", S_LO), ("dl_hi", S_HI),
                     ("sd_lo", S_LO), ("sd_hi", S_HI)):
        pk[name] = o
        o += w_
    pk["end"] = o

    return dict(
        splits=splits,
        tlo=tlo, thi=thi, off_lo=off_lo, off_hi=off_hi,
        S_LO=S_LO, S_HI=S_HI, batches=batches, pk=pk,
    )


def _wrap_idx(arr16):
    """[S*128] int16 -> gather idx layout [16, S*8] (i%16 part, i//16 col).
    The 8x replication across 16-partition groups happens on-device."""
    return arr16.reshape(-1, 16).T.copy()  # [16, S*8]


def _build_core_inputs(c, plan, src, dst, s_dst_node):
    s, e = plan["splits"][c], plan["splits"][c + 1]
    n0 = c * N_SH
    src_c = src[s:e]
    dst_c = dst[s:e]
    dloc = dst_c - n0
    win = dloc // W
    S_LO, S_HI = plan["S_LO"], plan["S_HI"]
    off_lo, off_hi = plan["off_lo"], plan["off_hi"]

    idx_lo = np.zeros(S_LO * TILE, dtype=np.int16)
    idx_hi = np.zeros(S_HI * TILE, dtype=np.int16)
    dstloc = np.full((S_LO + S_HI) * TILE, 999.0, dtype=_F16)
    sdst = np.zeros((S_LO + S_HI) * TILE, dtype=_F16)

    for half in ("lo", "hi"):
        mask = src_c < SPLIT if half == "lo" else src_c >= SPLIT
        ei = np.nonzero(mask)[0]
        w_e = win[ei]
        starts = np.searchsorted(w_e, np.arange(NW + 1))
        rank = np.arange(len(ei)) - starts[w_e]
        off = off_lo if half == "lo" else off_hi
        base = 0 if half == "lo" else S_LO * TILE
        flat = base + off[w_e] * TILE + rank
        dstloc[flat] = (dloc[ei] % W).astype(_F16)
        sdst[flat] = s_dst_node[dst_c[ei]]
        if half == "lo":
            idx_lo[off[w_e] * TILE + rank] = src_c[ei].astype(np.int16)
        else:
            idx_hi[off[w_e] * TILE + rank] = (src_c[ei] - SPLIT).astype(np.int16)

    dl2 = dstloc.reshape(S_LO + S_HI, TILE).T  # [128, S_LO+S_HI]
    sd2 = sdst.reshape(S_LO + S_HI, TILE).T

    pack_edge = np.empty((128, 2 * (S_LO + S_HI)), dtype=_F16)
    pack_edge[:, 0:S_LO] = dl2[:, :S_LO]
    pack_edge[:, S_LO:S_LO + S_HI] = dl2[:, S_LO:]
    pack_edge[:, S_LO + S_HI:2 * S_LO + S_HI] = sd2[:, :S_LO]
    pack_edge[:, 2 * S_LO + S_HI:] = sd2[:, S_LO:]

    pack16 = np.concatenate([_wrap_idx(idx_lo), _wrap_idx(idx_hi)], axis=1)
    return pack_edge, pack16


def _host_prep(h, src, dst, fc_w, attn_w):
    """Shared host-side preprocessing -> (plan, in_maps)."""
    plan = _plan(src, dst)

    a_src = attn_w[0, :OUT_DIM]
    a_dst = attn_w[0, OUT_DIM:]
    asrc_eff = fc_w.T @ a_src          # [128]
    adst_eff = fc_w.T @ a_dst          # [128]
    s_dst_node = (h @ adst_eff).astype(_F16)  # [N] host-side expansion data

    hrow = np.zeros((N_PAD, IN_DIM), dtype=_F16)
    hrow[:N_NODES] = h.astype(_F16)
    rhs_aug = np.concatenate(
        [asrc_eff[:, None], fc_w.T], axis=1).astype(_F16)  # [128, 65]
    iota_row = np.tile(np.arange(W, dtype=_F16)[None, :], (128, 1))
    pk = plan["pk"]

    common = np.empty((128, pk["dl_lo"]), dtype=_F16)
    common[:, pk["rhs"]:pk["rhs"] + 65] = rhs_aug
    common[:, pk["iota"]:pk["iota"] + W] = iota_row

    in_maps = []
    for c in range(NCORES):
        pack_edge, pack16 = _build_core_inputs(c, plan, src, dst, s_dst_node)
        pack = np.concatenate([common, pack_edge], axis=1)
        in_maps.append({"hsh": hrow[c * N_SH:(c + 1) * N_SH].view(np.int16),
                        "pack": pack.view(np.int16), "pack16": pack16})
    return plan, in_maps


# ----------------------------------------------------------------------
# Bass program
# ----------------------------------------------------------------------

def _build_bass(plan):
    import concourse.bacc as bacc
    import concourse.mybir as mybir
    import concourse.tile as tile

    f16 = mybir.dt.float16
    f32 = mybir.dt.float32
    i16 = mybir.dt.int16

    S_LO, S_HI = plan["S_LO"], plan["S_HI"]
    tlo, thi = plan["tlo"], plan["thi"]
    off_lo, off_hi = plan["off_lo"], plan["off_hi"]
    pk = plan["pk"]

    nc = bacc.Bacc("TRN2", target_bir_lowering=False, debug=False,
                   num_swdge_queues=4, dynamic_dma_scratch_size=SCRATCH)

    hsh_d = nc.declare_dram_parameter("hsh", [N_SH, IN_DIM], i16, isOutput=False)
    pack_d = nc.declare_dram_parameter("pack", [128, pk["end"]], i16, isOutput=False)
    pack16_d = nc.declare_dram_parameter("pack16", [16, (S_LO + S_HI) * 8], i16, isOutput=False)
    hout = nc.declare_dram_parameter("hout", [N_SH, OUT_DIM], f16, isOutput=True)
    packf = pack_d[:].bitcast(f16)

    hbounce = nc.dram_tensor("hbounce", [N_SH, IN_DIM], f16)
    table = nc.dram_tensor("htab", [N_PAD, IN_DIM], f16, addr_space="Shared")
    tabf = table[:]

    tab_lo = tabf[0:SPLIT, :]
    tab_hi = tabf[SPLIT:N_PAD, :]

    nbatch = int(os.environ.get("KERNEL_NBATCH", "0")) or len(plan["batches"])

    with tile.TileContext(nc) as tc:
        with (
            tc.tile_pool(name="sbB", bufs=3) as sbB,
            tc.tile_pool(name="sbP", bufs=5) as sbP,
            tc.tile_pool(name="sbBs", bufs=1) as sbBs,
            tc.tile_pool(name="sbC", bufs=3) as sbC,
            tc.tile_pool(name="psZ", bufs=2, space="PSUM") as psZ,
            tc.tile_pool(name="psW", bufs=2, space="PSUM") as psW,
        ):
            # bounce the h shard through SBUF into internal DRAM, then
            # AllGather the full row-major h table across the 8 cores.
            hb = sbBs.tile([128, N_SH], f16, tag="hb")
            nc.sync.dma_start(
                hb[:], hsh_d[:].bitcast(f16).rearrange(
                    "(a b) c -> a (b c)", a=128))
            nc.sync.dma_start(
                hbounce[:].rearrange("(a b) c -> a (b c)", a=128), hb[:])
            nc.gpsimd.collective_compute(
                "AllGather", mybir.AluOpType.bypass,
                replica_groups=[list(range(NCORES))],
                ins=[hbounce[:]], outs=[table[:]],
            )

            rhs_t = sbBs.tile([128, 65], f16, tag="rhs")
            nc.sync.dma_start(rhs_t[:], packf[:, pk["rhs"]:pk["rhs"] + 65])
            iota_t = sbBs.tile([128, W], f16, tag="iota")
            nc.sync.dma_start(iota_t[:], packf[:, pk["iota"]:pk["iota"] + W])
            # per-edge dst-local ids and s_dst, whole slot grid upfront
            dl_t = sbBs.tile([128, S_LO + S_HI], f16, tag="dl")
            nc.scalar.dma_start(dl_t[:, :S_LO], packf[:, pk["dl_lo"]:pk["dl_lo"] + S_LO])
            nc.scalar.dma_start(dl_t[:, S_LO:], packf[:, pk["dl_hi"]:pk["dl_hi"] + S_HI])
            sd_t = sbBs.tile([128, S_LO + S_HI], f16, tag="sd")
            nc.scalar.dma_start(sd_t[:, :S_LO], packf[:, pk["sd_lo"]:pk["sd_lo"] + S_LO])
            nc.scalar.dma_start(sd_t[:, S_LO:], packf[:, pk["sd_hi"]:pk["sd_hi"] + S_HI])
            # gather indices: ship [16, S*8] once, replicate into the 8
            # 16-partition groups on-device (SWDGE reads all 128 parts)
            it_t = sbBs.tile([128, (S_LO + S_HI) * 8], i16, tag="it")
            for g in range(8):
                nc.sync.dma_start(it_t[16 * g:16 * (g + 1), :], pack16_d[:])
            IT_HI0 = S_LO * 8  # column offset of hi idxs within it_t

            # Tile assigns DMASW sem lanes round-robin in emission order;
            # queue_num must track it so lane L always pairs queue L%4.
            gather_counter = [0]

            def batch_meta(bi):
                wins = plan["batches"][bi]
                alo = int(off_lo[wins[0]])
                ahi = int(off_hi[wins[0]])
                n_lo = int(off_lo[wins[-1] + 1]) - alo
                n_hi = int(off_hi[wins[-1] + 1]) - ahi
                return wins, alo, ahi, n_lo, n_hi

            # one-hot P depends only on dl_t/iota: emit builds PREB batches
            # ahead so DVE stays ahead of the PE/gather pipeline.
            PREB = 5
            P_tiles = {}

            def build_P(bi):
                _, alo, ahi, n_lo, n_hi = batch_meta(bi)
                n = n_lo + n_hi
                P = sbP.tile([128, n, W], f16, tag="P")
                nc.vector.tensor_tensor(
                    P[:, :n_lo, :],
                    dl_t[:, alo:alo + n_lo, None].to_broadcast([128, n_lo, W]),
                    iota_t[:, None, :].to_broadcast([128, n_lo, W]),
                    op=mybir.AluOpType.is_equal)
                nc.vector.tensor_tensor(
                    P[:, n_lo:, :],
                    dl_t[:, S_LO + ahi:S_LO + ahi + n_hi, None].to_broadcast(
                        [128, n_hi, W]),
                    iota_t[:, None, :].to_broadcast([128, n_hi, W]),
                    op=mybir.AluOpType.is_equal)
                P_tiles[bi] = P

            for bi in range(min(PREB, nbatch)):
                build_P(bi)

            for bi, wins in enumerate(plan["batches"][:nbatch]):
                w0 = wins[0]
                nb = len(wins)
                _, alo, ahi, n_lo, n_hi = batch_meta(bi)
                n = n_lo + n_hi

                # transposed gather: features on partitions, edges on free
                g2 = sbB.tile([128, n * TILE], f16, tag="g")
                for half, tab, a, nh, gbase, itbase in (
                    ("lo", tab_lo, alo, n_lo, 0, 0),
                    ("hi", tab_hi, ahi, n_hi, n_lo, IT_HI0),
                ):
                    for o in range(0, nh, GCAP):
                        k = min(GCAP, nh - o)
                        nc.gpsimd.dma_gather(
                            g2[:, (gbase + o) * TILE:(gbase + o + k) * TILE]
                            .rearrange("p (u e) -> p u e", u=1),
                            tab[:],
                            it_t[:, itbase + (a + o) * 8:itbase + (a + o + k) * 8],
                            num_idxs=k * TILE, num_idxs_reg=k * TILE,
                            elem_size=128, transpose=True,
                            queue_num=gather_counter[0] % NQ,
                        )
                        gather_counter[0] += 1

                # per-slot z matmul + PSUM->SBUF copy in ZGRP groups
                zc = sbB.tile([128, n, 65], f16, tag="zc")
                for z0 in range(0, n, ZGRP):
                    zn = min(ZGRP, n - z0)
                    Z = psZ.tile([128, ZGRP, 128], f32, tag="Z")
                    for si in range(zn):
                        nc.tensor.matmul(
                            Z[:, si, 0:65],
                            lhsT=g2[:, (z0 + si) * TILE:(z0 + si + 1) * TILE],
                            rhs=rhs_t[:], start=True, stop=True)
                    nc.scalar.copy(zc[:, z0:z0 + zn, :], Z[:, 0:zn, 0:65])

                # w = exp(leaky_relu(s_src + s_dst)); zc <- [w | w*z]
                e_t = sbB.tile([128, n], f16, tag="e")
                nc.vector.tensor_tensor(e_t[:, :n_lo], zc[:, :n_lo, 0],
                                        sd_t[:, alo:alo + n_lo],
                                        op=mybir.AluOpType.add)
                nc.vector.tensor_tensor(e_t[:, n_lo:], zc[:, n_lo:, 0],
                                        sd_t[:, S_LO + ahi:S_LO + ahi + n_hi],
                                        op=mybir.AluOpType.add)
                es = sbB.tile([128, n], f16, tag="es")
                nc.vector.tensor_scalar_mul(es[:], e_t[:], NEG_SLOPE)
                el = sbB.tile([128, n], f16, tag="el")
                nc.vector.tensor_tensor(el[:], e_t[:], es[:],
                                        op=mybir.AluOpType.max)
                wt = sbB.tile([128, n], f16, tag="w")
                nc.scalar.activation(wt[:], el[:],
                                     mybir.ActivationFunctionType.Exp)
                nc.scalar.copy(zc[:, :, 0:1], wt[:, :, None])
                nc.vector.tensor_tensor(
                    zc[:, :, 1:65], zc[:, :, 1:65],
                    wt[:, :, None].to_broadcast([128, n, 64]),
                    op=mybir.AluOpType.mult)
                P = P_tiles.pop(bi)

                ps = psW.tile([W, nb, 128], f32, tag="win")
                for wi, wv in enumerate(wins):
                    nmm = int(tlo[wv] + thi[wv])
                    k = 0
                    for j in range(int(tlo[wv])):
                        s_rel = int(off_lo[wv]) - alo + j
                        nc.tensor.matmul(ps[:, wi, 0:65], lhsT=P[:, s_rel, :],
                                         rhs=zc[:, s_rel, :],
                                         start=(k == 0), stop=(k == nmm - 1))
                        k += 1
                    for j in range(int(thi[wv])):
                        s_rel = n_lo + int(off_hi[wv]) - ahi + j
                        nc.tensor.matmul(ps[:, wi, 0:65], lhsT=P[:, s_rel, :],
                                         rhs=zc[:, s_rel, :],
                                         start=(k == 0), stop=(k == nmm - 1))
                        k += 1

                den = sbC.tile([W, nb], f32, tag="den")
                nc.vector.tensor_scalar_max(den[:], ps[:, :, 0], 1e-30)
                rec = sbC.tile([W, nb], f32, tag="rec")
                nc.vector.reciprocal(rec[:], den[:])
                ho = sbC.tile([W, nb * OUT_DIM], f16, tag="ho")
                nc.vector.tensor_tensor(
                    ho[:].rearrange("p (b c) -> p b c", b=nb),
                    ps[:, :, 1:65],
                    rec[:, :, None].to_broadcast([W, nb, OUT_DIM]),
                    op=mybir.AluOpType.mult)

                out_view = hout[w0 * W:(w0 + nb) * W, :].rearrange(
                    "(b p) c -> p b c", p=W)
                nc.sync.dma_start(
                    out_view,
                    ho[:].rearrange("p (b c) -> p b c", b=nb))

                if bi + PREB < nbatch:
                    build_P(bi + PREB)

    if not nc.is_finalized():
        nc.finalize()
    return nc


# ----------------------------------------------------------------------
# Entry point
# ----------------------------------------------------------------------

def kernel(h, src, dst, fc_w, attn_w):
    from concourse.bass_utils import run_bass_kernel_spmd

    h = np.asarray(h, dtype=_F32)
    src = np.asarray(src, dtype=np.int32)
    dst = np.asarray(dst, dtype=np.int32)
    fc_w = np.asarray(fc_w, dtype=_F32)
    attn_w = np.asarray(attn_w, dtype=_F32)

    plan, in_maps = _host_prep(h, src, dst, fc_w, attn_w)

    nc = _build_bass(plan)
    res = run_bass_kernel_spmd(nc, in_maps, list(range(NCORES)))
    global LAST_EXEC_NS
    LAST_EXEC_NS = res.exec_time_ns

    full = np.concatenate(
        [res.results[c]["hout"] for c in range(NCORES)], axis=0)
    return full[:N_NODES].astype(_F32)


# revision 4
# speedup vs baseline: 1.3228x; 1.1259x over previous
"""GAT layer (gnn_message_passing) Trainium2 Bass kernel, 8-core SPMD. v6

Strategy
--------
Nodes are partitioned contiguously across the 8 cores (6272 nodes each);
since dst is sorted, each core owns the contiguous run of edges whose dst
falls in its node range and computes those output rows fully locally.

Each core receives only its own 1/8 h shard (1.6MB); the full
row-major h table is assembled on-device with one AllGather into
Shared DRAM (real HW cost ~60us; shipping full h to every core instead
costs ~90MB of extra per-call input streaming through the axon relay,
which dominates wall time). There is NO z-table build phase: edges
gather raw h rows [N, 128] f16 from that table with
dma_gather(transpose=True), which lands FEATURES on partitions
([128, 1, E]). A per-slot PE matmul with the stationary
rhs_aug = [asrc_eff | fc_w.T] then computes Z = [s_src | z] for the
slot's 128 edges -- contraction over features puts EDGES back on
partitions, exactly what the downstream scatter needs. This replaces
the v1/v2 z-table in DRAM (25.6MB of build+write traffic and a
table-before-gather barrier) with 1.66 GFLOP/core of PE work.

HW-validated constraints for the transposed gather (found the hard
way): (a) all transposed gathers must ride ONE SWDGE queue --
concurrent transposed gathers on multiple queues race on the xbar and
corrupt ~15% of outputs; (b) num_idxs per call is capped at 896 (1024
hard-crashes the device); (c) negative gather indices fetch
nondeterministic garbage (zeros / wrong rows), so the int16 index
range forces the lo/hi table split at 32768.

Per batch (8 windows, ~70 slots):
  gather h_src (transposed)           [SWDGE, GCAP slots/call]
  per slot: Z[128e, 65] = g.T @ rhs   [PE, psZ 8-slot PSUM tiles]
  per 8-slot group: zc <- Z (f16)     [ACT copy PSUM->SBUF]
  e = zc[:,:,0] + s_dst; w = exp(lrelu(e))   [DVE + ACT exp]
  zc[:,:,0] <- w (ACT); zc[:,:,1:65] *= w    [DVE, in place]
  per slot: PSUM[W, wi, 0:65] += P_slot.T @ zc_slot   [PE]
  finalize per batch: den/recip/mul over all windows  [DVE]
with the one-hot P built on DVE (dl vs iota compare), PREB batches
ahead. Edge bookkeeping (dl, s_dst, int16 gather idx in lo/hi halves
split at 32768) is prepared on host and shipped in packed tensors.
"""

import os
import numpy as np

N_NODES = 50000
N_EDGES = 800000
IN_DIM = 128
OUT_DIM = 64
NEG_SLOPE = 0.01
NCORES = 8
W = 64           # nodes per window
TILE = 128       # edges per tile (= matmul contraction)
SPLIT = 32768    # int16 gather index limit
B_WIN = 8        # windows per batch
N_SH = 6272      # nodes per core (= 8*6272 = 50176 >= 50000)
N_PAD = NCORES * N_SH  # 50176
NW = N_SH // W   # 98 windows per core
ZGRP = 8         # slots per PSUM Z tile / ACT copy group
GCAP = int(os.environ.get("KERNEL_GCAP", "7"))   # slots per dma_gather
SCRATCH = int(os.environ.get("KERNEL_SCRATCH", "16384"))
NQ = int(os.environ.get("KERNEL_NQ", "1"))       # swdge queues used

_F16 = np.float16
_F32 = np.float32

LAST_EXEC_NS = None


# ----------------------------------------------------------------------
# Host planning
# ----------------------------------------------------------------------

def _plan(src, dst):
    splits = [int(np.searchsorted(dst, c * N_SH, side="left"))
              for c in range(NCORES + 1)]

    # per-window tile counts, shared across cores (SPMD: one NEFF)
    tlo = np.zeros(NW, dtype=np.int64)
    thi = np.zeros(NW, dtype=np.int64)
    for c in range(NCORES):
        s, e = splits[c], splits[c + 1]
        win = (dst[s:e] - c * N_SH) // W
        lo = src[s:e] < SPLIT
        wlo = np.bincount(win, weights=lo.astype(np.float64), minlength=NW).astype(np.int64)
        whi = np.bincount(win, minlength=NW) - wlo
        np.maximum(tlo, -(-wlo // TILE), out=tlo)
        np.maximum(thi, -(-whi // TILE), out=thi)
    tlo = np.maximum(tlo, 1)
    thi = np.maximum(thi, 1)

    off_lo = np.concatenate([[0], np.cumsum(tlo)])
    off_hi = np.concatenate([[0], np.cumsum(thi)])

    batches = [list(range(b, min(b + B_WIN, NW))) for b in range(0, NW, B_WIN)]

    S_LO, S_HI = int(off_lo[-1]), int(off_hi[-1])
    # packed [128, *] f16 column layout (no hT here -- h rides row-major
    # in its own "hrow" tensor for the transposed gather)
    pk = {}
    o = 0
    for name, w_ in (("rhs", 65), ("iota", W),
                     ("dl_lo", S_LO), ("dl_hi", S_HI),
                     ("sd_lo", S_LO), ("sd_hi", S_HI)):
        pk[name] = o
        o += w_
    pk["end"] = o

    return dict(
        splits=splits,
        tlo=tlo, thi=thi, off_lo=off_lo, off_hi=off_hi,
        S_LO=S_LO, S_HI=S_HI, batches=batches, pk=pk,
    )


def _wrap_idx(arr16):
    """[S*128] int16 -> gather idx layout [16, S*8] (i%16 part, i//16 col).
    The 8x replication across 16-partition groups happens on-device."""
    return arr16.reshape(-1, 16).T.copy()  # [16, S*8]


def _build_core_inputs(c, plan, src, dst, s_dst_node):
    s, e = plan["splits"][c], plan["splits"][c + 1]
    n0 = c * N_SH
    src_c = src[s:e]
    dst_c = dst[s:e]
    dloc = dst_c - n0
    win = dloc // W
    S_LO, S_HI = plan["S_LO"], plan["S_HI"]
    off_lo, off_hi = plan["off_lo"], plan["off_hi"]

    idx_lo = np.zeros(S_LO * TILE, dtype=np.int16)
    idx_hi = np.zeros(S_HI * TILE, dtype=np.int16)
    dstloc = np.full((S_LO + S_HI) * TILE, 999.0, dtype=_F16)
    sdst = np.zeros((S_LO + S_HI) * TILE, dtype=_F16)

    for half in ("lo", "hi"):
        mask = src_c < SPLIT if half == "lo" else src_c >= SPLIT
        ei = np.nonzero(mask)[0]
        w_e = win[ei]
        starts = np.searchsorted(w_e, np.arange(NW + 1))
        rank = np.arange(len(ei)) - starts[w_e]
        off = off_lo if half == "lo" else off_hi
        base = 0 if half == "lo" else S_LO * TILE
        flat = base + off[w_e] * TILE + rank
        dstloc[flat] = (dloc[ei] % W).astype(_F16)
        sdst[flat] = s_dst_node[dst_c[ei]]
        if half == "lo":
            idx_lo[off[w_e] * TILE + rank] = src_c[ei].astype(np.int16)
        else:
            idx_hi[off[w_e] * TILE + rank] = (src_c[ei] - SPLIT).astype(np.int16)

    dl2 = dstloc.reshape(S_LO + S_HI, TILE).T  # [128, S_LO+S_HI]
    sd2 = sdst.reshape(S_LO + S_HI, TILE).T

    pack_edge = np.empty((128, 2 * (S_LO + S_HI)), dtype=_F16)
    pack_edge[:, 0:S_LO] = dl2[:, :S_LO]
    pack_edge[:, S_LO:S_LO + S_HI] = dl2[:, S_LO:]
    pack_edge[:, S_LO + S_HI:2 * S_LO + S_HI] = sd2[:, :S_LO]
    pack_edge[:, 2 * S_LO + S_HI:] = sd2[:, S_LO:]

    pack16 = np.concatenate([_wrap_idx(idx_lo), _wrap_idx(idx_hi)], axis=1)
    return pack_edge, pack16


def _host_prep(h, src, dst, fc_w, attn_w):
    """Shared host-side preprocessing -> (plan, in_maps)."""
    plan = _plan(src, dst)

    a_src = attn_w[0, :OUT_DIM]
    a_dst = attn_w[0, OUT_DIM:]
    asrc_eff = fc_w.T @ a_src          # [128]
    adst_eff = fc_w.T @ a_dst          # [128]
    s_dst_node = (h @ adst_eff).astype(_F16)  # [N] host-side expansion data

    hrow = np.zeros((N_PAD, IN_DIM), dtype=_F16)
    hrow[:N_NODES] = h.astype(_F16)
    rhs_aug = np.concatenate(
        [asrc_eff[:, None], fc_w.T], axis=1).astype(_F16)  # [128, 65]
    iota_row = np.tile(np.arange(W, dtype=_F16)[None, :], (128, 1))
    pk = plan["pk"]

    common = np.empty((128, pk["dl_lo"]), dtype=_F16)
    common[:, pk["rhs"]:pk["rhs"] + 65] = rhs_aug
    common[:, pk["iota"]:pk["iota"] + W] = iota_row

    in_maps = []
    for c in range(NCORES):
        pack_edge, pack16 = _build_core_inputs(c, plan, src, dst, s_dst_node)
        pack = np.concatenate([common, pack_edge], axis=1)
        in_maps.append({"hsh": hrow[c * N_SH:(c + 1) * N_SH].view(np.int16),
                        "pack": pack.view(np.int16), "pack16": pack16})
    return plan, in_maps


# ----------------------------------------------------------------------
# Bass program
# ----------------------------------------------------------------------

def _build_bass(plan):
    import concourse.bacc as bacc
    import concourse.mybir as mybir
    import concourse.tile as tile

    f16 = mybir.dt.float16
    f32 = mybir.dt.float32
    i16 = mybir.dt.int16

    S_LO, S_HI = plan["S_LO"], plan["S_HI"]
    tlo, thi = plan["tlo"], plan["thi"]
    off_lo, off_hi = plan["off_lo"], plan["off_hi"]
    pk = plan["pk"]

    nc = bacc.Bacc("TRN2", target_bir_lowering=False, debug=False,
                   num_swdge_queues=4, dynamic_dma_scratch_size=SCRATCH)

    hsh_d = nc.declare_dram_parameter("hsh", [N_SH, IN_DIM], i16, isOutput=False)
    pack_d = nc.declare_dram_parameter("pack", [128, pk["end"]], i16, isOutput=False)
    pack16_d = nc.declare_dram_parameter("pack16", [16, (S_LO + S_HI) * 8], i16, isOutput=False)
    hout = nc.declare_dram_parameter("hout", [N_SH, OUT_DIM], f16, isOutput=True)
    packf = pack_d[:].bitcast(f16)

    hbounce = nc.dram_tensor("hbounce", [N_SH, IN_DIM], f16)
    table = nc.dram_tensor("htab", [N_PAD, IN_DIM], f16, addr_space="Shared")
    tabf = table[:]

    tab_lo = tabf[0:SPLIT, :]
    tab_hi = tabf[SPLIT:N_PAD, :]

    nbatch = int(os.environ.get("KERNEL_NBATCH", "0")) or len(plan["batches"])

    with tile.TileContext(nc) as tc:
        with (
            tc.tile_pool(name="sbB", bufs=3) as sbB,
            tc.tile_pool(name="sbP", bufs=5) as sbP,
            tc.tile_pool(name="sbBs", bufs=1) as sbBs,
            tc.tile_pool(name="sbC", bufs=3) as sbC,
            tc.tile_pool(name="psZ", bufs=2, space="PSUM") as psZ,
            tc.tile_pool(name="psW", bufs=2, space="PSUM") as psW,
        ):
            # bounce the h shard through SBUF into internal DRAM, then
            # AllGather the full row-major h table across the 8 cores.
            hb = sbBs.tile([128, N_SH], f16, tag="hb")
            nc.sync.dma_start(
                hb[:], hsh_d[:].bitcast(f16).rearrange(
                    "(a b) c -> a (b c)", a=128))
            nc.sync.dma_start(
                hbounce[:].rearrange("(a b) c -> a (b c)", a=128), hb[:])
            nc.gpsimd.collective_compute(
                "AllGather", mybir.AluOpType.bypass,
                replica_groups=[list(range(NCORES))],
                ins=[hbounce[:]], outs=[table[:]],
            )

            rhs_t = sbBs.tile([128, 65], f16, tag="rhs")
            nc.sync.dma_start(rhs_t[:], packf[:, pk["rhs"]:pk["rhs"] + 65])
            iota_t = sbBs.tile([128, W], f16, tag="iota")
            nc.sync.dma_start(iota_t[:], packf[:, pk["iota"]:pk["iota"] + W])
            # per-edge dst-local ids and s_dst, whole slot grid upfront
            dl_t = sbBs.tile([128, S_LO + S_HI], f16, tag="dl")
            nc.scalar.dma_start(dl_t[:, :S_LO], packf[:, pk["dl_lo"]:pk["dl_lo"] + S_LO])
            nc.scalar.dma_start(dl_t[:, S_LO:], packf[:, pk["dl_hi"]:pk["dl_hi"] + S_HI])
            sd_t = sbBs.tile([128, S_LO + S_HI], f16, tag="sd")
            nc.scalar.dma_start(sd_t[:, :S_LO], packf[:, pk["sd_lo"]:pk["sd_lo"] + S_LO])
            nc.scalar.dma_start(sd_t[:, S_LO:], packf[:, pk["sd_hi"]:pk["sd_hi"] + S_HI])
            # gather indices: ship [16, S*8] once, replicate into the 8
            # 16-partition groups on-device (SWDGE reads all 128 parts)
            it_t = sbBs.tile([128, (S_LO + S_HI) * 8], i16, tag="it")
            for g in range(8):
                nc.sync.dma_start(it_t[16 * g:16 * (g + 1), :], pack16_d[:])
            IT_HI0 = S_LO * 8  # column offset of hi idxs within it_t

            # Tile assigns DMASW sem lanes round-robin in emission order;
            # queue_num must track it so lane L always pairs queue L%4.
            gather_counter = [0]

            def batch_meta(bi):
                wins = plan["batches"][bi]
                alo = int(off_lo[wins[0]])
                ahi = int(off_hi[wins[0]])
                n_lo = int(off_lo[wins[-1] + 1]) - alo
                n_hi = int(off_hi[wins[-1] + 1]) - ahi
                return wins, alo, ahi, n_lo, n_hi

            # one-hot P depends only on dl_t/iota: emit builds PREB batches
            # ahead so DVE stays ahead of the PE/gather pipeline.
            PREB = 5
            P_tiles = {}

            def build_P(bi):
                _, alo, ahi, n_lo, n_hi = batch_meta(bi)
                n = n_lo + n_hi
                P = sbP.tile([128, n, W], f16, tag="P")
                nc.vector.tensor_tensor(
                    P[:, :n_lo, :],
                    dl_t[:, alo:alo + n_lo, None].to_broadcast([128, n_lo, W]),
                    iota_t[:, None, :].to_broadcast([128, n_lo, W]),
                    op=mybir.AluOpType.is_equal)
                nc.vector.tensor_tensor(
                    P[:, n_lo:, :],
                    dl_t[:, S_LO + ahi:S_LO + ahi + n_hi, None].to_broadcast(
                        [128, n_hi, W]),
                    iota_t[:, None, :].to_broadcast([128, n_hi, W]),
                    op=mybir.AluOpType.is_equal)
                P_tiles[bi] = P

            for bi in range(min(PREB, nbatch)):
                build_P(bi)

            for bi, wins in enumerate(plan["batches"][:nbatch]):
                w0 = wins[0]
                nb = len(wins)
                _, alo, ahi, n_lo, n_hi = batch_meta(bi)
                n = n_lo + n_hi

                # transposed gather: features on partitions, edges on free
                g2 = sbB.tile([128, n * TILE], f16, tag="g")
                for half, tab, a, nh, gbase, itbase in (
                    ("lo", tab_lo, alo, n_lo, 0, 0),
                    ("hi", tab_hi, ahi, n_hi, n_lo, IT_HI0),
                ):
                    for o in range(0, nh, GCAP):
                        k = min(GCAP, nh - o)
                        nc.gpsimd.dma_gather(
                            g2[:, (gbase + o) * TILE:(gbase + o + k) * TILE]
                            .rearrange("p (u e) -> p u e", u=1),
                            tab[:],
                            it_t[:, itbase + (a + o) * 8:itbase + (a + o + k) * 8],
                            num_idxs=k * TILE, num_idxs_reg=k * TILE,
                            elem_size=128, transpose=True,
                            queue_num=gather_counter[0] % NQ,
                        )
                        gather_counter[0] += 1

                # per-slot z matmul + PSUM->SBUF copy in ZGRP groups
                zc = sbB.tile([128, n, 65], f16, tag="zc")
                for z0 in range(0, n, ZGRP):
                    zn = min(ZGRP, n - z0)
                    Z = psZ.tile([128, ZGRP, 128], f32, tag="Z")
                    for si in range(zn):
                        nc.tensor.matmul(
                            Z[:, si, 0:65],
                            lhsT=g2[:, (z0 + si) * TILE:(z0 + si + 1) * TILE],
                            rhs=rhs_t[:], start=True, stop=True)
                    nc.scalar.copy(zc[:, z0:z0 + zn, :], Z[:, 0:zn, 0:65])

                # w = exp(leaky_relu(s_src + s_dst)); zc <- [w | w*z]
                e_t = sbB.tile([128, n], f16, tag="e")
                nc.vector.tensor_tensor(e_t[:, :n_lo], zc[:, :n_lo, 0],
                                        sd_t[:, alo:alo + n_lo],
                                        op=mybir.AluOpType.add)
                nc.vector.tensor_tensor(e_t[:, n_lo:], zc[:, n_lo:, 0],
                                        sd_t[:, S_LO + ahi:S_LO + ahi + n_hi],
                                        op=mybir.AluOpType.add)
                es = sbB.tile([128, n], f16, tag="es")
                nc.vector.tensor_scalar_mul(es[:], e_t[:], NEG_SLOPE)
                el = sbB.tile([128, n], f16, tag="el")
                nc.vector.tensor_tensor(el[:], e_t[:], es[:],
                                        op=mybir.AluOpType.max)
                wt = sbB.tile([128, n], f16, tag="w")
                nc.scalar.activation(wt[:], el[:],
                                     mybir.ActivationFunctionType.Exp)
                nc.scalar.copy(zc[:, :, 0:1], wt[:, :, None])
                nc.vector.tensor_tensor(
                    zc[:, :, 1:65], zc[:, :, 1:65],
                    wt[:, :, None].to_broadcast([128, n, 64]),
                    op=mybir.AluOpType.mult)
                P = P_tiles.pop(bi)

                ps = psW.tile([W, nb, 128], f32, tag="win")
                for wi, wv in enumerate(wins):
                    nmm = int(tlo[wv] + thi[wv])
                    k = 0
                    for j in range(int(tlo[wv])):
                        s_rel = int(off_lo[wv]) - alo + j
                        nc.tensor.matmul(ps[:, wi, 0:65], lhsT=P[:, s_rel, :],
                                         rhs=zc[:, s_rel, :],
                                         start=(k == 0), stop=(k == nmm - 1))
                        k += 1
                    for j in range(int(thi[wv])):
                        s_rel = n_lo + int(off_hi[wv]) - ahi + j
                        nc.tensor.matmul(ps[:, wi, 0:65], lhsT=P[:, s_rel, :],
                                         rhs=zc[:, s_rel, :],
                                         start=(k == 0), stop=(k == nmm - 1))
                        k += 1

                den = sbC.tile([W, nb], f32, tag="den")
                nc.vector.tensor_scalar_max(den[:], ps[:, :, 0], 1e-30)
                rec = sbC.tile([W, nb], f32, tag="rec")
                nc.vector.reciprocal(rec[:], den[:])
                ho = sbC.tile([W, nb * OUT_DIM], f16, tag="ho")
                nc.vector.tensor_tensor(
                    ho[:].rearrange("p (b c) -> p b c", b=nb),
                    ps[:, :, 1:65],
                    rec[:, :, None].to_broadcast([W, nb, OUT_DIM]),
                    op=mybir.AluOpType.mult)

                out_view = hout[w0 * W:(w0 + nb) * W, :].rearrange(
                    "(b p) c -> p b c", p=W)
                nc.sync.dma_start(
                    out_view,
                    ho[:].rearrange("p (b c) -> p b c", b=nb))

                if bi + PREB < nbatch:
                    build_P(bi + PREB)

    if not nc.is_finalized():
        nc.finalize()
    return nc


# ----------------------------------------------------------------------
# Entry point
# ----------------------------------------------------------------------

def kernel(h, src, dst, fc_w, attn_w):
    from concourse.bass_utils import run_bass_kernel_spmd

    h = np.asarray(h, dtype=_F32)
    src = np.asarray(src, dtype=np.int32)
    dst = np.asarray(dst, dtype=np.int32)
    fc_w = np.asarray(fc_w, dtype=_F32)
    attn_w = np.asarray(attn_w, dtype=_F32)

    plan, in_maps = _host_prep(h, src, dst, fc_w, attn_w)

    nc = _build_bass(plan)
    res = run_bass_kernel_spmd(nc, in_maps, list(range(NCORES)))
    global LAST_EXEC_NS
    LAST_EXEC_NS = res.exec_time_ns

    full = np.concatenate(
        [res.results[c]["hout"] for c in range(NCORES)], axis=0)
    return full[:N_NODES].astype(_F32)
